# revision 24
# baseline (speedup 1.0000x reference)
"""TRN2 Bass/Tile kernel for nn_Attention (B=4, H=16, S=2048, D=64, fp32).

Entry point: kernel(q, k, v) -> out, all full-shape [4, 16, 2048, 64] fp32.

Sharding: batch*heads = 64 head-slices, 8 per NeuronCore (data/head
parallel, no cross-core communication). Each core runs the same NEFF on
its own 8 slices via run_bass_kernel_spmd.

v2 design. The v1 baseline was softmax-exp bound: exp on the scalar
(ACT) engine is 1 elem/cycle/partition @1.2GHz = ~218us/core of ACT time
vs ~180us of PE time, measured 379-439us with imperfect overlap. v2:

  - S^T formulation: QK^T row-packed matmuls (two 64-contraction halves
    at tile_position (0,0)/(64,0)) produce S^T[j,i] half-tiles [128,512]
    in PSUM; PV accumulates O_aug^T[65, i] += Vtilde_c^T @ expS^T_c over
    j-chunks (Vtilde = [V | ones]; row 64 = softmax denominator).
  - qT/kT are bf16 (scores' error ~0.3% of sigma -- negligible), built
    with zero PE/DVE work: Pool (gpsimd) converts f32->bf16, then
    XBAR DMA-transposes write each chunk's [64,128] transpose directly
    into the qT/kT layouts. PE runs *only* QK and PV matmuls.
  - exp is split across engines per [128,512] half-step:
      * ACT halves: true Exp activation (scale=1/8 folded in).
      * DVE halves: Schraudolph-style bitcast exponential, sum of two
        affine-int32 terms (2-segment piecewise-linear, rel err ~±1%):
          u  = int32(A*qk + B1)      (DVE tensor_scalar, PSUM->SBUF)
          w  = u - D                 (Pool int sub, SBUF->SBUF)
          et = f32(u) + f32(w)       (Pool tensor_tensor add)
        Per-head numpy sim of the mixed softmax: rel err ~6e-3.
  - software-pipelined emission: QK runs LOOKAHEAD half-steps ahead of
    PV so the in-order PE queue never waits on exp; next head's
    loads/converts/transposes are emitted mid-head.
  - epilogue per i-group: O_aug^T -> bf16 -> XBAR DMA-transpose ->
    [128, 4, 80]; batched DVE reciprocal + broadcast multiply.
  - PSUM: 6 banks of QK half-tiles + 2 banks of PV accumulators.

This container's walrus build rejects sync waits on Drain instructions
and allows at most one sync wait on any other instruction, while Tile
freely attaches several; _patch_tile_framework() + _split_sync_waits()
below rework the exit barrier and hoist excess waits onto injected NOPs.
"""
import sys

if '/opt/trn_rl_repo' not in sys.path:
    sys.path.insert(0, '/opt/trn_rl_repo')

import numpy as np

import concourse.bass as bass
import concourse.tile as tile
from concourse import mybir
from concourse.masks import make_identity
from concourse.vector_clock import ScopedClock

F32 = mybir.dt.float32
F32R = mybir.dt.float32r
BF16 = mybir.dt.bfloat16
I32 = mybir.dt.int32
EXP = mybir.ActivationFunctionType.Exp
ALU = mybir.AluOpType

B, H, S, D = 4, 16, 2048, 64
N_CORES = 8
HEADS_PER_CORE = B * H // N_CORES

# --- exp approximation constants (see module docstring) -------------------
LOG2E = 1.4426950408889634
LA1 = -0.8263                  # tuned: minimax of S(x,la1)+S(x,la2) vs exp
LA2 = -1.3163
A_SCALE = LOG2E * (1 << 23) * 0.125       # folds the 1/sqrt(D) score scale
B1_CONST = (127.0 + LA1) * (1 << 23)
D_SUB = round((LA1 - LA2) * (1 << 23))

# --- engine assignment knobs ---------------------------------------------
N_HALVES = 64           # half-steps per head: (i-groups=4) x (j-pairs=8) x 2
ACT_HALVES = 41         # halves whose exp runs on ACT; rest on DVE+Pool
USE_POOL_EXP = False    # TS2+TT on gpsimd (False: on DVE)
USE_POOL_CVT = True     # preamble vt convert on gpsimd (False: on DVE)
USE_DMA_TR = False      # XBAR DMA transposes (False: PE transposes)
SKIP_EXP = False        # timing-only: PV reads one static et; exp lanes idle


def _assignment():
    """Bresenham-spread booleans: True = ACT half-step."""
    return [
        (t + 1) * ACT_HALVES // N_HALVES - t * ACT_HALVES // N_HALVES == 1
        for t in range(N_HALVES)
    ]


# ---------------------------------------------------------------------------
# Walrus compatibility patches
# ---------------------------------------------------------------------------
_patched = False
_split_counter = [0]


def _patched_multi_engine_barrier(self, engines):
    for e in engines:
        self.engines[e].drain(fusable=False)
    for inst in self._sem_only_all_engine_barrier_insts(f"aeb{self.next_id()}"):
        self.engines[inst.engine].add_instruction(inst)


def _patched_drain_and_barrier(self, tick_clock, wait_clock):
    nop_inst = self.nc.sync.nop(nofuse=True, hint="tile_exit_wait")
    wait_clock.add_sem_waits(
        nop_inst.ins, ScopedClock({None: tick_clock.global_clock})
    )
    self.nc.sync.drain()
    self.nc.all_engine_barrier()
    assert self.sems is not None
    popped = self.nc._tile_sem_poison_stack.pop()
    assert popped is self._sem_poison
    self.nc.clear_and_free_semaphores(list(self.sems.allocated().values()))
    self.nc.all_engine_barrier()


def _patch_tile_framework():
    global _patched
    if _patched:
        return
    bass.Bass.multi_engine_barrier = _patched_multi_engine_barrier
    tile.TileContext._drain_and_barrier = _patched_drain_and_barrier
    _patched = True


def _split_sync_waits(nc):
    """No instruction may carry more than the walrus-supported number of
    sync waits (0 for Drain, 1 otherwise); hoist the rest onto NOPs."""
    for f in nc.m.functions:
        for bb in f.blocks:
            insts = bb.instructions
            if not any(
                i.sync_info is not None
                and len(i.sync_info.on_wait) > (0 if i.opcode == "Drain" else 1)
                for i in insts
            ):
                continue
            out = []
            for inst in insts:
                si = inst.sync_info
                limit = 0 if inst.opcode == "Drain" else 1
                if si is not None and len(si.on_wait) > limit:
                    waits = list(si.on_wait)
                    keep, extra = waits[:limit], waits[limit:]
                    for w in extra:
                        _split_counter[0] += 1
                        nop = mybir.InstNoOp(
                            name=f"waitsplit-{_split_counter[0]}", ins=[], outs=[]
                        )
                        nop.engine = inst.engine
                        nop.sync_info = mybir.SyncInfo(on_wait=[w], on_update=[])
                        out.append(nop)
                    inst.sync_info = mybir.SyncInfo(
                        on_wait=keep, on_update=list(si.on_update)
                    )
                out.append(inst)
            bb.instructions = out


# ---------------------------------------------------------------------------
# Kernel builder
# ---------------------------------------------------------------------------
def build_nc(heads=HEADS_PER_CORE, s=S, reps=1, walrus_compat=True):
    NJ = s // 128           # j (k-row) chunks of 128
    IG = 512                # i (q-row) group width
    NG = s // IG
    NT = IG // 128
    NH = NG * NJ            # half-steps per head (== N_HALVES for s=2048)
    LOOKP = 3 if USE_DMA_TR else 2  # QK lookahead in cc-pairs

    acts = _assignment()

    nc = bass.Bass(target_bir_lowering=False)
    q_d = nc.dram_tensor("q", [heads, s, D], F32, kind="ExternalInput")
    k_d = nc.dram_tensor("k", [heads, s, D], F32, kind="ExternalInput")
    v_d = nc.dram_tensor("v", [heads, s, D], F32, kind="ExternalInput")
    o_d = nc.dram_tensor("o", [heads, s, D], F32, kind="ExternalOutput")

    with tile.TileContext(nc) as tc:
        with (
            tc.tile_pool(name="qkin", bufs=2) as qkin,
            tc.tile_pool(name="qkT", bufs=2) as qkT,
            tc.tile_pool(name="exps", bufs=8) as exps,
            tc.tile_pool(name="uw", bufs=4) as uwp,
            tc.tile_pool(name="osb", bufs=2) as osb,
            tc.tile_pool(name="singles", bufs=1) as singles,
            tc.tile_pool(name="qkps", bufs=6 if USE_DMA_TR else 5,
                         space="PSUM") as qkps,
            tc.tile_pool(name="pvps", bufs=2, space="PSUM") as pvps,
            tc.tile_pool(name="trin", bufs=1, space="PSUM") as trin,
        ):
            if not USE_DMA_TR:
                ident = singles.tile([128, 128], F32)
                make_identity(nc, ident)
                identr = singles.tile([128, 128], F32R)
                nc.vector.tensor_copy(identr, ident)
            def emit_preamble(h):
                """Loads + bf16 transposed layouts for head h via Pool
                converts and XBAR DMA-transposes. Returns (qT, kT, vl)."""
                ceng = nc.gpsimd if USE_POOL_CVT else nc.vector
                qn = qkin.tile([128, NJ, D], F32, tag="qn")
                kn = qkin.tile([128, NJ, D], F32, tag="kn")
                nc.sync.dma_start(
                    out=qn, in_=q_d[h].rearrange("(c p) d -> p c d", p=128))
                nc.sync.dma_start(
                    out=kn, in_=k_d[h].rearrange("(c p) d -> p c d", p=128))
                vl = qkin.tile([128, NJ, 80], F32, tag="vl")
                nc.sync.dma_start(
                    out=vl[:, :, 0:D],
                    in_=v_d[h].rearrange("(c p) d -> p c d", p=128))
                nc.vector.memset(vl[:, :, D:D + 1], 1.0)
                nc.vector.memset(vl[:, :, D + 1:80], 0.0)
                vt = qkin.tile([128, NJ, 80], F32R, tag="vt")
                ceng.tensor_copy(vt, vl)

                kT = qkT.tile([128, s // 2], BF16, tag="kT")
                qTp = qkT.tile([128, s // 2], BF16, tag="qTp")
                if USE_DMA_TR:
                    # XBAR-transpose chunk PAIRS ([128,128] slabs; free dim
                    # must be a multiple of 128). Output rows 0:64 = even
                    # chunk, 64:128 = odd -- exactly the kT pair layout.
                    qb = qkin.tile([128, NJ, D], BF16, tag="qb")
                    kb = qkin.tile([128, NJ, D], BF16, tag="kb")
                    ceng.tensor_copy(qb, qn)
                    ceng.tensor_copy(kb, kn)
                    for pp in range(NJ // 2):
                        nc.sync.dma_start_transpose(
                            kT[:, pp * 128:(pp + 1) * 128],
                            kb[:, 2 * pp:2 * pp + 2, :].rearrange(
                                "p a b -> p (a b)"))
                    for pp in range(NJ // 2):
                        nc.sync.dma_start_transpose(
                            qTp[:, pp * 128:(pp + 1) * 128],
                            qb[:, 2 * pp:2 * pp + 2, :].rearrange(
                                "p a b -> p (a b)"))
                else:
                    # PE pair-transposes (bf16 identity streams at 1 c/row;
                    # f32r data passes through exactly), DVE copies convert
                    # the f32r PSUM result to bf16.
                    for src_, dst in ((kn, kT), (qn, qTp)):
                        for hf in range(2):
                            tk = trin.tile([128, 512], F32, tag="tr",
                                           name="tk")
                            for i in range(4):
                                pp = hf * 4 + i
                                nc.tensor.transpose(
                                    tk[:, i * 128:(i + 1) * 128],
                                    src_[:, 2 * pp:2 * pp + 2, :].rearrange(
                                        "p a b -> p (a b)"),
                                    ident)
                            nc.vector.tensor_copy(
                                dst[:, hf * 512:(hf + 1) * 512], tk)
                # qTp[a*64+d, pp*128+p] = q[(2pp+a)*128+p, d]; reassemble to
                # qT[d, c*128+p] = q[c*128+p, d] on both partition halves.
                qT = qkT.tile([128, s], BF16, tag="qT")
                nc.sync.dma_start(
                    out=qT[0:64].rearrange(
                        "p (a c) -> p a c", a=NJ // 2)[:, :, 0:128],
                    in_=qTp[0:64].rearrange("p (a c) -> p a c", a=NJ // 2))
                nc.sync.dma_start(
                    out=qT[0:64].rearrange(
                        "p (a c) -> p a c", a=NJ // 2)[:, :, 128:256],
                    in_=qTp[64:128].rearrange("p (a c) -> p a c", a=NJ // 2))
                nc.sync.dma_start(out=qT[64:128], in_=qT[0:64])
                return qT, kT, vt

            def body():
                ctx = {0: emit_preamble(0)}

                for h in range(heads):
                    qT, kT, vt = ctx.pop(h)
                    ps_t, et_t, pv_t = {}, {}, {}
                    if SKIP_EXP:
                        et_fix = exps.tile([128, IG], F32R, tag="et",
                                           name="et_fix")
                        nc.vector.tensor_copy(
                            et_fix,
                            vt.rearrange("p a b -> p (a b)")[:, 0:IG])

                    def emit_qk(m):
                        # half-step m: g = m//(2*NG... flat: step n = m//2
                        n, half = divmod(m, 2)
                        g, cc = divmod(n, NJ // 2)
                        ps = qkps.tile([128, IG], F32, tag="ps", name="ps")
                        ps_t[m] = ps
                        nc.tensor.matmul(
                            ps,
                            kT[half * 64:half * 64 + 64,
                               cc * 128:(cc + 1) * 128],
                            qT[half * 64:half * 64 + 64,
                               g * IG:(g + 1) * IG],
                            start=True, stop=True,
                            tile_position=(half * 64, 0))

                    def emit_exp(m):
                        if SKIP_EXP:
                            ps_t.pop(m)
                            et_t[m] = et_fix
                            return
                        ps = ps_t.pop(m)
                        et = exps.tile([128, IG], F32R, tag="et", name="et")
                        et_t[m] = et
                        if acts[m % N_HALVES]:
                            nc.scalar.activation(et, ps, EXP, scale=0.125)
                        else:
                            u = uwp.tile([128, IG], F32, tag="u", name="u")
                            w = uwp.tile([128, IG], F32, tag="w", name="w")
                            nc.vector.tensor_scalar(
                                u.bitcast(I32), ps, A_SCALE, B1_CONST,
                                ALU.mult, ALU.add)
                            eng = nc.gpsimd if USE_POOL_EXP else nc.vector
                            eng.tensor_scalar_sub(
                                w.bitcast(I32), u.bitcast(I32), D_SUB)
                            eng.tensor_tensor(et, u, w, ALU.add)

                    def emit_pv(m):
                        g = m // (2 * (NJ // 2))
                        c = m % NJ          # j-chunk index
                        if c == 0:
                            pv_t[g] = pvps.tile(
                                [80, IG], F32, tag="pv", name="pv")
                        et = et_t.pop(m)
                        nc.tensor.matmul(
                            pv_t[g],
                            vt[:, c, :],
                            et,
                            start=(c == 0), stop=(c == NJ - 1))

                    def emit_epilogue(g):
                        pv = pv_t.pop(g)
                        if USE_DMA_TR:
                            og = osb.tile([80, IG], BF16, tag="og")
                            nc.vector.tensor_copy(og, pv)
                            ot = osb.tile([128, NT, 80], BF16, tag="ot")
                            for t in range(NT):
                                nc.sync.dma_start_transpose(
                                    ot[:, t, :], og[:, t * 128:(t + 1) * 128])
                            rcin = ot[:, :, D:D + 1]
                            ooin = ot[:, :, 0:D]
                        else:
                            og = osb.tile([80, IG], F32R, tag="og")
                            nc.vector.tensor_copy(og, pv)
                            ot = trin.tile([128, NT, 80], F32R, tag="tr",
                                           name="ot")
                            for t in range(NT):
                                nc.tensor.transpose(
                                    ot[:, t, :],
                                    og[:, t * 128:(t + 1) * 128],
                                    identr[0:80, 0:80])
                            rcin = ot[:, :, D:D + 1].bitcast(F32)
                            ooin = ot[:, :, 0:D].bitcast(F32)
                        rc = osb.tile([128, NT, 1], F32, tag="rc")
                        nc.vector.reciprocal(rc, rcin)
                        oo = osb.tile([128, NT, D], F32, tag="oo")
                        nc.vector.tensor_tensor(
                            oo, ooin,
                            rc.broadcast_to([128, NT, D]), ALU.mult)
                        nc.sync.dma_start(
                            out=o_d[h, g * IG:(g + 1) * IG, :].rearrange(
                                "(t p) d -> p t d", p=128),
                            in_=oo)

                    # software pipeline over cc-PAIRS: the two row-tiled QK
                    # halves are emitted adjacently so they co-execute in the
                    # PE array (row-groups 0-1 vs 2-3; measured ~2x).
                    NP = NH // 2
                    for p in range(min(LOOKP, NP)):
                        emit_qk(2 * p)
                        emit_qk(2 * p + 1)
                        if p < LOOKP - 1:
                            emit_exp(2 * p)
                            emit_exp(2 * p + 1)
                    for p in range(NP):
                        emit_pv(2 * p)
                        emit_pv(2 * p + 1)
                        if p + LOOKP < NP:
                            emit_qk(2 * (p + LOOKP))
                            emit_qk(2 * (p + LOOKP) + 1)
                        if p + LOOKP - 1 < NP:
                            emit_exp(2 * (p + LOOKP - 1))
                            emit_exp(2 * (p + LOOKP - 1) + 1)
                        if p == NP // 2 and h + 1 < heads:
                            ctx[h + 1] = emit_preamble(h + 1)
                        if p % (NJ // 2) == NJ // 2 - 1:
                            emit_epilogue(p // (NJ // 2))

            if reps == 1:
                body()
            else:
                with tc.For_i(0, reps, 1):
                    body()

    if walrus_compat:
        _split_sync_waits(nc)
    return nc


_cached_nc = None


def _get_nc():
    global _cached_nc
    if _cached_nc is None:
        _patch_tile_framework()
        _cached_nc = build_nc()
    return _cached_nc


def kernel(q, k, v):
    """Full-shape attention: q/k/v [4, 16, 2048, 64] fp32 -> same shape."""
    from concourse.bass_utils import run_bass_kernel_spmd

    nc = _get_nc()
    q = np.ascontiguousarray(np.asarray(q, dtype=np.float32)).reshape(B * H, S, D)
    k = np.ascontiguousarray(np.asarray(k, dtype=np.float32)).reshape(B * H, S, D)
    v = np.ascontiguousarray(np.asarray(v, dtype=np.float32)).reshape(B * H, S, D)
    hpc = HEADS_PER_CORE
    in_maps = [
        {"q": q[i * hpc:(i + 1) * hpc],
         "k": k[i * hpc:(i + 1) * hpc],
         "v": v[i * hpc:(i + 1) * hpc]}
        for i in range(N_CORES)
    ]
    res = run_bass_kernel_spmd(nc, in_maps, core_ids=list(range(N_CORES)))
    out = np.concatenate([res.results[i]["o"] for i in range(N_CORES)], axis=0)
    return out.reshape(B, H, S, D)


# revision 26
# speedup vs baseline: 1.1443x; 1.1443x over previous
"""TRN2 Bass/Tile kernel for nn_Attention (B=4, H=16, S=2048, D=64, fp32).

Entry point: kernel(q, k, v) -> out, all full-shape [4, 16, 2048, 64] fp32.

Sharding: batch*heads = 64 head-slices, 8 per NeuronCore (data/head
parallel, no cross-core communication). Each core runs the same NEFF on
its own 8 slices via run_bass_kernel_spmd.

v2 design. The v1 baseline was softmax-exp bound: exp on the scalar
(ACT) engine is 1 elem/cycle/partition @1.2GHz = ~218us/core of ACT time
vs ~180us of PE time, measured 379-439us with imperfect overlap. v2:

  - S^T formulation: QK^T row-packed matmuls (two 64-contraction halves
    at tile_position (0,0)/(64,0)) produce S^T[j,i] half-tiles [128,512]
    in PSUM; PV accumulates O_aug^T[65, i] += Vtilde_c^T @ expS^T_c over
    j-chunks (Vtilde = [V | ones]; row 64 = softmax denominator).
  - qT/kT are bf16 (scores' error ~0.3% of sigma -- negligible), built
    with zero PE/DVE work: Pool (gpsimd) converts f32->bf16, then
    XBAR DMA-transposes write each chunk's [64,128] transpose directly
    into the qT/kT layouts. PE runs *only* QK and PV matmuls.
  - exp is split across engines per [128,512] half-step:
      * ACT halves: true Exp activation (scale=1/8 folded in).
      * DVE halves: Schraudolph-style bitcast exponential, sum of two
        affine-int32 terms (2-segment piecewise-linear, rel err ~±1%):
          u  = int32(A*qk + B1)      (DVE tensor_scalar, PSUM->SBUF)
          w  = u - D                 (Pool int sub, SBUF->SBUF)
          et = f32(u) + f32(w)       (Pool tensor_tensor add)
        Per-head numpy sim of the mixed softmax: rel err ~6e-3.
  - software-pipelined emission: QK runs LOOKAHEAD half-steps ahead of
    PV so the in-order PE queue never waits on exp; next head's
    loads/converts/transposes are emitted mid-head.
  - epilogue per i-group: O_aug^T -> bf16 -> XBAR DMA-transpose ->
    [128, 4, 80]; batched DVE reciprocal + broadcast multiply.
  - PSUM: 6 banks of QK half-tiles + 2 banks of PV accumulators.

This container's walrus build rejects sync waits on Drain instructions
and allows at most one sync wait on any other instruction, while Tile
freely attaches several; _patch_tile_framework() + _split_sync_waits()
below rework the exit barrier and hoist excess waits onto injected NOPs.
"""
import sys

if '/opt/trn_rl_repo' not in sys.path:
    sys.path.insert(0, '/opt/trn_rl_repo')

import numpy as np

import concourse.bass as bass
import concourse.tile as tile
from concourse import mybir
from concourse.masks import make_identity
from concourse.vector_clock import ScopedClock

F32 = mybir.dt.float32
F32R = mybir.dt.float32r
BF16 = mybir.dt.bfloat16
I32 = mybir.dt.int32
EXP = mybir.ActivationFunctionType.Exp
ALU = mybir.AluOpType

B, H, S, D = 4, 16, 2048, 64
N_CORES = 8
HEADS_PER_CORE = B * H // N_CORES

# --- exp approximation constants (see module docstring) -------------------
LOG2E = 1.4426950408889634
LA1 = -0.8263                  # tuned: minimax of S(x,la1)+S(x,la2) vs exp
LA2 = -1.3163
A_SCALE = LOG2E * (1 << 23) * 0.125       # folds the 1/sqrt(D) score scale
B1_CONST = (127.0 + LA1) * (1 << 23)
D_SUB = round((LA1 - LA2) * (1 << 23))

# --- engine assignment knobs ---------------------------------------------
N_HALVES = 64           # half-steps per head: (i-groups=4) x (j-pairs=8) x 2
ACT_HALVES = 41         # halves whose exp runs on ACT; rest on DVE+Pool
USE_POOL_EXP = False    # TS2+TT on gpsimd (False: on DVE)
USE_POOL_CVT = True     # preamble vt convert on gpsimd (False: on DVE)
USE_DMA_TR = False      # XBAR DMA transposes (False: PE transposes)
SKIP_EXP = False        # timing-only: PV reads one static et; exp lanes idle


def _assignment():
    """Bresenham-spread booleans: True = ACT half-step."""
    return [
        (t + 1) * ACT_HALVES // N_HALVES - t * ACT_HALVES // N_HALVES == 1
        for t in range(N_HALVES)
    ]


# ---------------------------------------------------------------------------
# Walrus compatibility patches
# ---------------------------------------------------------------------------
_patched = False
_split_counter = [0]


def _patched_multi_engine_barrier(self, engines):
    for e in engines:
        self.engines[e].drain(fusable=False)
    for inst in self._sem_only_all_engine_barrier_insts(f"aeb{self.next_id()}"):
        self.engines[inst.engine].add_instruction(inst)


def _patched_drain_and_barrier(self, tick_clock, wait_clock):
    nop_inst = self.nc.sync.nop(nofuse=True, hint="tile_exit_wait")
    wait_clock.add_sem_waits(
        nop_inst.ins, ScopedClock({None: tick_clock.global_clock})
    )
    self.nc.sync.drain()
    self.nc.all_engine_barrier()
    assert self.sems is not None
    popped = self.nc._tile_sem_poison_stack.pop()
    assert popped is self._sem_poison
    self.nc.clear_and_free_semaphores(list(self.sems.allocated().values()))
    self.nc.all_engine_barrier()


def _patch_tile_framework():
    global _patched
    if _patched:
        return
    bass.Bass.multi_engine_barrier = _patched_multi_engine_barrier
    tile.TileContext._drain_and_barrier = _patched_drain_and_barrier
    _patched = True


def _split_sync_waits(nc):
    """No instruction may carry more than the walrus-supported number of
    sync waits (0 for Drain, 1 otherwise); hoist the rest onto NOPs."""
    for f in nc.m.functions:
        for bb in f.blocks:
            insts = bb.instructions
            if not any(
                i.sync_info is not None
                and len(i.sync_info.on_wait) > (0 if i.opcode == "Drain" else 1)
                for i in insts
            ):
                continue
            out = []
            for inst in insts:
                si = inst.sync_info
                limit = 0 if inst.opcode == "Drain" else 1
                if si is not None and len(si.on_wait) > limit:
                    waits = list(si.on_wait)
                    keep, extra = waits[:limit], waits[limit:]
                    for w in extra:
                        _split_counter[0] += 1
                        nop = mybir.InstNoOp(
                            name=f"waitsplit-{_split_counter[0]}", ins=[], outs=[]
                        )
                        nop.engine = inst.engine
                        nop.sync_info = mybir.SyncInfo(on_wait=[w], on_update=[])
                        out.append(nop)
                    inst.sync_info = mybir.SyncInfo(
                        on_wait=keep, on_update=list(si.on_update)
                    )
                out.append(inst)
            bb.instructions = out


# ---------------------------------------------------------------------------
# Kernel builder
# ---------------------------------------------------------------------------
def build_nc(heads=HEADS_PER_CORE, s=S, reps=1, walrus_compat=True):
    NJ = s // 128           # j (k-row) chunks of 128
    IG = 512                # i (q-row) group width
    NG = s // IG
    NT = IG // 128
    NH = NG * NJ            # half-steps per head (== N_HALVES for s=2048)
    LOOKP = 3               # QK lookahead in cc-pairs

    acts = _assignment()

    nc = bass.Bass(target_bir_lowering=False)
    q_d = nc.dram_tensor("q", [heads, s, D], F32, kind="ExternalInput")
    k_d = nc.dram_tensor("k", [heads, s, D], F32, kind="ExternalInput")
    v_d = nc.dram_tensor("v", [heads, s, D], F32, kind="ExternalInput")
    o_d = nc.dram_tensor("o", [heads, s, D], F32, kind="ExternalOutput")

    with tile.TileContext(nc) as tc:
        with (
            tc.tile_pool(name="qkin", bufs=2) as qkin,
            tc.tile_pool(name="qkT", bufs=2) as qkT,
            tc.tile_pool(name="exps", bufs=8) as exps,
            tc.tile_pool(name="uw", bufs=4) as uwp,
            tc.tile_pool(name="osb", bufs=2) as osb,
            tc.tile_pool(name="singles", bufs=1) as singles,
            tc.tile_pool(name="qkps", bufs=6, space="PSUM") as qkps,
            tc.tile_pool(name="pvps", bufs=2, space="PSUM") as pvps,
        ):
            if not USE_DMA_TR:
                ident = singles.tile([128, 128], F32)
                make_identity(nc, ident)
                identr = singles.tile([128, 128], F32R)
                nc.vector.tensor_copy(identr, ident)
            def emit_preamble(h):
                """Loads + bf16 transposed layouts for head h via Pool
                converts and XBAR DMA-transposes. Returns (qT, kT, vl)."""
                ceng = nc.gpsimd if USE_POOL_CVT else nc.vector
                qn = qkin.tile([128, NJ, D], F32, tag="qn")
                kn = qkin.tile([128, NJ, D], F32, tag="kn")
                nc.sync.dma_start(
                    out=qn, in_=q_d[h].rearrange("(c p) d -> p c d", p=128))
                nc.sync.dma_start(
                    out=kn, in_=k_d[h].rearrange("(c p) d -> p c d", p=128))
                vl = qkin.tile([128, NJ, 80], F32, tag="vl")
                nc.sync.dma_start(
                    out=vl[:, :, 0:D],
                    in_=v_d[h].rearrange("(c p) d -> p c d", p=128))
                nc.vector.memset(vl[:, :, D:D + 1], 1.0)
                nc.vector.memset(vl[:, :, D + 1:80], 0.0)
                vt = qkin.tile([128, NJ, 80], F32R, tag="vt")
                ceng.tensor_copy(vt, vl)

                kT = qkT.tile([128, s // 2], BF16, tag="kT")
                if USE_DMA_TR:
                    qTp = qkT.tile([128, s // 2], BF16, tag="qTp")
                    # XBAR-transpose chunk PAIRS ([128,128] slabs; free dim
                    # must be a multiple of 128). Output rows 0:64 = even
                    # chunk, 64:128 = odd -- exactly the kT pair layout.
                    qb = qkin.tile([128, NJ, D], BF16, tag="qb")
                    kb = qkin.tile([128, NJ, D], BF16, tag="kb")
                    ceng.tensor_copy(qb, qn)
                    ceng.tensor_copy(kb, kn)
                    for pp in range(NJ // 2):
                        nc.sync.dma_start_transpose(
                            kT[:, pp * 128:(pp + 1) * 128],
                            kb[:, 2 * pp:2 * pp + 2, :].rearrange(
                                "p a b -> p (a b)"))
                    for pp in range(NJ // 2):
                        nc.sync.dma_start_transpose(
                            qTp[:, pp * 128:(pp + 1) * 128],
                            qb[:, 2 * pp:2 * pp + 2, :].rearrange(
                                "p a b -> p (a b)"))
                    # qTp[a*64+d, pp*128+p] = q[(2pp+a)*128+p, d]; reshuffle
                    # into qT[d, c*128+p] = q[c*128+p, d], both halves.
                    qT = qkT.tile([128, s], BF16, tag="qT")
                    nc.sync.dma_start(
                        out=qT[0:64].rearrange(
                            "p (a c) -> p a c", a=NJ // 2)[:, :, 0:128],
                        in_=qTp[0:64].rearrange("p (a c) -> p a c", a=NJ // 2))
                    nc.sync.dma_start(
                        out=qT[0:64].rearrange(
                            "p (a c) -> p a c", a=NJ // 2)[:, :, 128:256],
                        in_=qTp[64:128].rearrange(
                            "p (a c) -> p a c", a=NJ // 2))
                    nc.sync.dma_start(out=qT[64:128], in_=qT[0:64])
                    return qT, kT, vt
                # PE path. K pair-transposes land in the kT pair layout
                # directly (Pool converts PSUM f32 -> bf16); Q single-chunk
                # transposes ([128,64] -> [64,128]) write qT[0:64] directly
                # (DVE converts), then one DMA duplicates the partition
                # halves. Transpose staging tiles live in the fast-rotating
                # qkps PSUM pool.
                for hf in range(2):
                    tk = qkps.tile([128, 512], F32, tag="ps", name="tk")
                    for i in range(4):
                        pp = hf * 4 + i
                        nc.tensor.transpose(
                            tk[:, i * 128:(i + 1) * 128],
                            kn[:, 2 * pp:2 * pp + 2, :].rearrange(
                                "p a b -> p (a b)"),
                            ident)
                    nc.vector.tensor_copy(
                        kT[:, hf * 512:(hf + 1) * 512], tk)
                qT = qkT.tile([128, s], BF16, tag="qT")
                for quad in range(4):
                    tq = qkps.tile([128, 512], F32, tag="ps", name="tq")
                    for i in range(4):
                        c = quad * 4 + i
                        nc.tensor.transpose(
                            tq[0:64, i * 128:(i + 1) * 128],
                            qn[:, c, :], ident)
                    nc.vector.tensor_copy(
                        qT[0:64, quad * 512:(quad + 1) * 512], tq[0:64, :])
                nc.sync.dma_start(out=qT[64:128], in_=qT[0:64])
                return qT, kT, vt

            def body():
                ctx = {0: emit_preamble(0)}

                for h in range(heads):
                    qT, kT, vt = ctx.pop(h)
                    ps_t, et_t, pv_t = {}, {}, {}
                    if SKIP_EXP:
                        et_fix = exps.tile([128, IG], F32R, tag="et",
                                           name="et_fix")
                        nc.vector.tensor_copy(
                            et_fix,
                            vt.rearrange("p a b -> p (a b)")[:, 0:IG])

                    def emit_qk(m):
                        # half-step m: g = m//(2*NG... flat: step n = m//2
                        n, half = divmod(m, 2)
                        g, cc = divmod(n, NJ // 2)
                        ps = qkps.tile([128, IG], F32, tag="ps", name="ps")
                        ps_t[m] = ps
                        nc.tensor.matmul(
                            ps,
                            kT[half * 64:half * 64 + 64,
                               cc * 128:(cc + 1) * 128],
                            qT[half * 64:half * 64 + 64,
                               g * IG:(g + 1) * IG],
                            start=True, stop=True,
                            tile_position=(half * 64, 0))

                    def emit_exp(m):
                        if SKIP_EXP:
                            ps_t.pop(m)
                            et_t[m] = et_fix
                            return
                        ps = ps_t.pop(m)
                        et = exps.tile([128, IG], F32R, tag="et", name="et")
                        et_t[m] = et
                        if acts[m % N_HALVES]:
                            nc.scalar.activation(et, ps, EXP, scale=0.125)
                        else:
                            u = uwp.tile([128, IG], F32, tag="u", name="u")
                            w = uwp.tile([128, IG], F32, tag="w", name="w")
                            nc.vector.tensor_scalar(
                                u.bitcast(I32), ps, A_SCALE, B1_CONST,
                                ALU.mult, ALU.add)
                            eng = nc.gpsimd if USE_POOL_EXP else nc.vector
                            eng.tensor_scalar_sub(
                                w.bitcast(I32), u.bitcast(I32), D_SUB)
                            eng.tensor_tensor(et, u, w, ALU.add)

                    def emit_pv(m):
                        g = m // (2 * (NJ // 2))
                        c = m % NJ          # j-chunk index
                        if c == 0:
                            pv_t[g] = pvps.tile(
                                [80, IG], F32, tag="pv", name="pv")
                        et = et_t.pop(m)
                        nc.tensor.matmul(
                            pv_t[g],
                            vt[:, c, :],
                            et,
                            start=(c == 0), stop=(c == NJ - 1))

                    def emit_epilogue(g):
                        pv = pv_t.pop(g)
                        if USE_DMA_TR:
                            og = osb.tile([80, IG], BF16, tag="og")
                            nc.vector.tensor_copy(og, pv)
                            ot = osb.tile([128, NT, 80], BF16, tag="ot")
                            for t in range(NT):
                                nc.sync.dma_start_transpose(
                                    ot[:, t, :], og[:, t * 128:(t + 1) * 128])
                            rcin = ot[:, :, D:D + 1]
                            ooin = ot[:, :, 0:D]
                        else:
                            og = osb.tile([80, IG], F32R, tag="og")
                            nc.vector.tensor_copy(og, pv)
                            ot = qkps.tile([128, NT, 80], F32R, tag="ps",
                                           name="ot")
                            for t in range(NT):
                                nc.tensor.transpose(
                                    ot[:, t, :],
                                    og[:, t * 128:(t + 1) * 128],
                                    identr[0:80, 0:80])
                            rcin = ot[:, :, D:D + 1].bitcast(F32)
                            ooin = ot[:, :, 0:D].bitcast(F32)
                        rc = osb.tile([128, NT, 1], F32, tag="rc")
                        nc.vector.reciprocal(rc, rcin)
                        oo = osb.tile([128, NT, D], F32, tag="oo")
                        nc.vector.tensor_tensor(
                            oo, ooin,
                            rc.broadcast_to([128, NT, D]), ALU.mult)
                        nc.sync.dma_start(
                            out=o_d[h, g * IG:(g + 1) * IG, :].rearrange(
                                "(t p) d -> p t d", p=128),
                            in_=oo)

                    # software pipeline over cc-PAIRS: the two row-tiled QK
                    # halves are emitted adjacently so they co-execute in the
                    # PE array (row-groups 0-1 vs 2-3; measured ~2x).
                    NP = NH // 2
                    for p in range(min(LOOKP, NP)):
                        emit_qk(2 * p)
                        emit_qk(2 * p + 1)
                        if p < LOOKP - 1:
                            emit_exp(2 * p)
                            emit_exp(2 * p + 1)
                    for p in range(NP):
                        emit_pv(2 * p)
                        emit_pv(2 * p + 1)
                        if p + LOOKP < NP:
                            emit_qk(2 * (p + LOOKP))
                            emit_qk(2 * (p + LOOKP) + 1)
                        if p + LOOKP - 1 < NP:
                            emit_exp(2 * (p + LOOKP - 1))
                            emit_exp(2 * (p + LOOKP - 1) + 1)
                        if p == NP // 2 and h + 1 < heads:
                            ctx[h + 1] = emit_preamble(h + 1)
                        if p % (NJ // 2) == NJ // 2 - 1:
                            emit_epilogue(p // (NJ // 2))

            if reps == 1:
                body()
            else:
                with tc.For_i(0, reps, 1):
                    body()

    if walrus_compat:
        _split_sync_waits(nc)
    return nc


_cached_nc = None


def _get_nc():
    global _cached_nc
    if _cached_nc is None:
        _patch_tile_framework()
        _cached_nc = build_nc()
    return _cached_nc


def kernel(q, k, v):
    """Full-shape attention: q/k/v [4, 16, 2048, 64] fp32 -> same shape."""
    from concourse.bass_utils import run_bass_kernel_spmd

    nc = _get_nc()
    q = np.ascontiguousarray(np.asarray(q, dtype=np.float32)).reshape(B * H, S, D)
    k = np.ascontiguousarray(np.asarray(k, dtype=np.float32)).reshape(B * H, S, D)
    v = np.ascontiguousarray(np.asarray(v, dtype=np.float32)).reshape(B * H, S, D)
    hpc = HEADS_PER_CORE
    in_maps = [
        {"q": q[i * hpc:(i + 1) * hpc],
         "k": k[i * hpc:(i + 1) * hpc],
         "v": v[i * hpc:(i + 1) * hpc]}
        for i in range(N_CORES)
    ]
    res = run_bass_kernel_spmd(nc, in_maps, core_ids=list(range(N_CORES)))
    out = np.concatenate([res.results[i]["o"] for i in range(N_CORES)], axis=0)
    return out.reshape(B, H, S, D)


# revision 27
# speedup vs baseline: 1.1654x; 1.0185x over previous
"""TRN2 Bass/Tile kernel for nn_Attention (B=4, H=16, S=2048, D=64, fp32).

Entry point: kernel(q, k, v) -> out, all full-shape [4, 16, 2048, 64] fp32.

Sharding: batch*heads = 64 head-slices, 8 per NeuronCore (data/head
parallel, no cross-core communication). Each core runs the same NEFF on
its own 8 slices via run_bass_kernel_spmd.

v2 design. The v1 baseline was softmax-exp bound: exp on the scalar
(ACT) engine is 1 elem/cycle/partition @1.2GHz = ~218us/core of ACT time
vs ~180us of PE time, measured 379-439us with imperfect overlap. v2:

  - S^T formulation: QK^T row-packed matmuls (two 64-contraction halves
    at tile_position (0,0)/(64,0)) produce S^T[j,i] half-tiles [128,512]
    in PSUM; PV accumulates O_aug^T[65, i] += Vtilde_c^T @ expS^T_c over
    j-chunks (Vtilde = [V | ones]; row 64 = softmax denominator).
  - qT/kT are bf16 (scores' error ~0.3% of sigma -- negligible), built
    with zero PE/DVE work: Pool (gpsimd) converts f32->bf16, then
    XBAR DMA-transposes write each chunk's [64,128] transpose directly
    into the qT/kT layouts. PE runs *only* QK and PV matmuls.
  - exp is split across engines per [128,512] half-step:
      * ACT halves: true Exp activation (scale=1/8 folded in).
      * DVE halves: Schraudolph-style bitcast exponential, sum of two
        affine-int32 terms (2-segment piecewise-linear, rel err ~±1%):
          u  = int32(A*qk + B1)      (DVE tensor_scalar, PSUM->SBUF)
          w  = u - D                 (Pool int sub, SBUF->SBUF)
          et = f32(u) + f32(w)       (Pool tensor_tensor add)
        Per-head numpy sim of the mixed softmax: rel err ~6e-3.
  - software-pipelined emission: QK runs LOOKAHEAD half-steps ahead of
    PV so the in-order PE queue never waits on exp; next head's
    loads/converts/transposes are emitted mid-head.
  - epilogue per i-group: O_aug^T -> bf16 -> XBAR DMA-transpose ->
    [128, 4, 80]; batched DVE reciprocal + broadcast multiply.
  - PSUM: 6 banks of QK half-tiles + 2 banks of PV accumulators.

This container's walrus build rejects sync waits on Drain instructions
and allows at most one sync wait on any other instruction, while Tile
freely attaches several; _patch_tile_framework() + _split_sync_waits()
below rework the exit barrier and hoist excess waits onto injected NOPs.
"""
import sys

if '/opt/trn_rl_repo' not in sys.path:
    sys.path.insert(0, '/opt/trn_rl_repo')

import numpy as np

import concourse.bass as bass
import concourse.tile as tile
from concourse import mybir
from concourse.masks import make_identity
from concourse.vector_clock import ScopedClock

F32 = mybir.dt.float32
F32R = mybir.dt.float32r
BF16 = mybir.dt.bfloat16
I32 = mybir.dt.int32
EXP = mybir.ActivationFunctionType.Exp
ALU = mybir.AluOpType

B, H, S, D = 4, 16, 2048, 64
N_CORES = 8
HEADS_PER_CORE = B * H // N_CORES

# --- exp approximation constants (see module docstring) -------------------
LOG2E = 1.4426950408889634
LA1 = -0.8263                  # tuned: minimax of S(x,la1)+S(x,la2) vs exp
LA2 = -1.3163
A_SCALE = LOG2E * (1 << 23) * 0.125       # folds the 1/sqrt(D) score scale
B1_CONST = (127.0 + LA1) * (1 << 23)
D_SUB = round((LA1 - LA2) * (1 << 23))

# --- engine assignment knobs ---------------------------------------------
N_HALVES = 64           # half-steps per head: (i-groups=4) x (j-pairs=8) x 2
ACT_HALVES = 41         # halves whose exp runs on ACT; rest on DVE+Pool
USE_POOL_EXP = False    # TS2+TT on gpsimd (False: on DVE)
USE_POOL_CVT = True     # preamble vt convert on gpsimd (False: on DVE)
USE_DMA_TR = False      # XBAR DMA transposes (False: PE transposes)
SKIP_EXP = False        # timing-only: PV reads one static et; exp lanes idle


def _assignment():
    """Bresenham-spread booleans: True = ACT half-step."""
    return [
        (t + 1) * ACT_HALVES // N_HALVES - t * ACT_HALVES // N_HALVES == 1
        for t in range(N_HALVES)
    ]


# ---------------------------------------------------------------------------
# Walrus compatibility patches
# ---------------------------------------------------------------------------
_patched = False
_split_counter = [0]


def _patched_multi_engine_barrier(self, engines):
    for e in engines:
        self.engines[e].drain(fusable=False)
    for inst in self._sem_only_all_engine_barrier_insts(f"aeb{self.next_id()}"):
        self.engines[inst.engine].add_instruction(inst)


def _patched_drain_and_barrier(self, tick_clock, wait_clock):
    nop_inst = self.nc.sync.nop(nofuse=True, hint="tile_exit_wait")
    wait_clock.add_sem_waits(
        nop_inst.ins, ScopedClock({None: tick_clock.global_clock})
    )
    self.nc.sync.drain()
    self.nc.all_engine_barrier()
    assert self.sems is not None
    popped = self.nc._tile_sem_poison_stack.pop()
    assert popped is self._sem_poison
    self.nc.clear_and_free_semaphores(list(self.sems.allocated().values()))
    self.nc.all_engine_barrier()


def _patch_tile_framework():
    global _patched
    if _patched:
        return
    bass.Bass.multi_engine_barrier = _patched_multi_engine_barrier
    tile.TileContext._drain_and_barrier = _patched_drain_and_barrier
    _patched = True


def _split_sync_waits(nc):
    """No instruction may carry more than the walrus-supported number of
    sync waits (0 for Drain, 1 otherwise); hoist the rest onto NOPs."""
    for f in nc.m.functions:
        for bb in f.blocks:
            insts = bb.instructions
            if not any(
                i.sync_info is not None
                and len(i.sync_info.on_wait) > (0 if i.opcode == "Drain" else 1)
                for i in insts
            ):
                continue
            out = []
            for inst in insts:
                si = inst.sync_info
                limit = 0 if inst.opcode == "Drain" else 1
                if si is not None and len(si.on_wait) > limit:
                    waits = list(si.on_wait)
                    keep, extra = waits[:limit], waits[limit:]
                    for w in extra:
                        _split_counter[0] += 1
                        nop = mybir.InstNoOp(
                            name=f"waitsplit-{_split_counter[0]}", ins=[], outs=[]
                        )
                        nop.engine = inst.engine
                        nop.sync_info = mybir.SyncInfo(on_wait=[w], on_update=[])
                        out.append(nop)
                    inst.sync_info = mybir.SyncInfo(
                        on_wait=keep, on_update=list(si.on_update)
                    )
                out.append(inst)
            bb.instructions = out


# ---------------------------------------------------------------------------
# Kernel builder
# ---------------------------------------------------------------------------
def build_nc(heads=HEADS_PER_CORE, s=S, reps=1, walrus_compat=True):
    NJ = s // 128           # j (k-row) chunks of 128
    IG = 512                # i (q-row) group width
    NG = s // IG
    NT = IG // 128
    NH = NG * NJ            # half-steps per head (== N_HALVES for s=2048)
    LOOKP = 2               # QK lookahead in cc-pairs

    acts = _assignment()

    nc = bass.Bass(target_bir_lowering=False)
    q_d = nc.dram_tensor("q", [heads, s, D], F32, kind="ExternalInput")
    k_d = nc.dram_tensor("k", [heads, s, D], F32, kind="ExternalInput")
    v_d = nc.dram_tensor("v", [heads, s, D], F32, kind="ExternalInput")
    o_d = nc.dram_tensor("o", [heads, s, D], F32, kind="ExternalOutput")

    with tile.TileContext(nc) as tc:
        with (
            tc.tile_pool(name="qkin", bufs=2) as qkin,
            tc.tile_pool(name="qkT", bufs=2) as qkT,
            tc.tile_pool(name="exps", bufs=8) as exps,
            tc.tile_pool(name="uw", bufs=4) as uwp,
            tc.tile_pool(name="osb", bufs=2) as osb,
            tc.tile_pool(name="singles", bufs=1) as singles,
            tc.tile_pool(name="qkps", bufs=5, space="PSUM") as qkps,
            tc.tile_pool(name="pvps", bufs=2, space="PSUM") as pvps,
            tc.tile_pool(name="trin", bufs=1, space="PSUM") as trin,
        ):
            if not USE_DMA_TR:
                ident = singles.tile([128, 128], F32)
                make_identity(nc, ident)
                identr = singles.tile([128, 128], F32R)
                nc.vector.tensor_copy(identr, ident)
            def emit_preamble(h):
                """Loads + bf16 transposed layouts for head h via Pool
                converts and XBAR DMA-transposes. Returns (qT, kT, vl)."""
                ceng = nc.gpsimd if USE_POOL_CVT else nc.vector
                qn = qkin.tile([128, NJ, D], F32, tag="qn")
                kn = qkin.tile([128, NJ, D], F32, tag="kn")
                nc.sync.dma_start(
                    out=qn, in_=q_d[h].rearrange("(c p) d -> p c d", p=128))
                nc.sync.dma_start(
                    out=kn, in_=k_d[h].rearrange("(c p) d -> p c d", p=128))
                vl = qkin.tile([128, NJ, 80], F32, tag="vl")
                nc.sync.dma_start(
                    out=vl[:, :, 0:D],
                    in_=v_d[h].rearrange("(c p) d -> p c d", p=128))
                nc.vector.memset(vl[:, :, D:D + 1], 1.0)
                nc.vector.memset(vl[:, :, D + 1:80], 0.0)
                vt = qkin.tile([128, NJ, 80], F32R, tag="vt")
                ceng.tensor_copy(vt, vl)

                kT = qkT.tile([128, s // 2], BF16, tag="kT")
                if USE_DMA_TR:
                    qTp = qkT.tile([128, s // 2], BF16, tag="qTp")
                    # XBAR-transpose chunk PAIRS ([128,128] slabs; free dim
                    # must be a multiple of 128). Output rows 0:64 = even
                    # chunk, 64:128 = odd -- exactly the kT pair layout.
                    qb = qkin.tile([128, NJ, D], BF16, tag="qb")
                    kb = qkin.tile([128, NJ, D], BF16, tag="kb")
                    ceng.tensor_copy(qb, qn)
                    ceng.tensor_copy(kb, kn)
                    for pp in range(NJ // 2):
                        nc.sync.dma_start_transpose(
                            kT[:, pp * 128:(pp + 1) * 128],
                            kb[:, 2 * pp:2 * pp + 2, :].rearrange(
                                "p a b -> p (a b)"))
                    for pp in range(NJ // 2):
                        nc.sync.dma_start_transpose(
                            qTp[:, pp * 128:(pp + 1) * 128],
                            qb[:, 2 * pp:2 * pp + 2, :].rearrange(
                                "p a b -> p (a b)"))
                    # qTp[a*64+d, pp*128+p] = q[(2pp+a)*128+p, d]; reshuffle
                    # into qT[d, c*128+p] = q[c*128+p, d], both halves.
                    qT = qkT.tile([128, s], BF16, tag="qT")
                    nc.sync.dma_start(
                        out=qT[0:64].rearrange(
                            "p (a c) -> p a c", a=NJ // 2)[:, :, 0:128],
                        in_=qTp[0:64].rearrange("p (a c) -> p a c", a=NJ // 2))
                    nc.sync.dma_start(
                        out=qT[0:64].rearrange(
                            "p (a c) -> p a c", a=NJ // 2)[:, :, 128:256],
                        in_=qTp[64:128].rearrange(
                            "p (a c) -> p a c", a=NJ // 2))
                    nc.sync.dma_start(out=qT[64:128], in_=qT[0:64])
                    return qT, kT, vt
                # PE path. K pair-transposes land in the kT pair layout
                # directly (Pool converts PSUM f32 -> bf16); Q single-chunk
                # transposes ([128,64] -> [64,128]) write qT[0:64] directly
                # (DVE converts), then one DMA duplicates the partition
                # halves. Transpose staging tiles live in the fast-rotating
                # qkps PSUM pool.
                for hf in range(2):
                    tk = trin.tile([128, 512], F32, tag="tr", name="tk")
                    for i in range(4):
                        pp = hf * 4 + i
                        nc.tensor.transpose(
                            tk[:, i * 128:(i + 1) * 128],
                            kn[:, 2 * pp:2 * pp + 2, :].rearrange(
                                "p a b -> p (a b)"),
                            ident)
                    nc.vector.tensor_copy(
                        kT[:, hf * 512:(hf + 1) * 512], tk)
                qT = qkT.tile([128, s], BF16, tag="qT")
                for quad in range(4):
                    tq = trin.tile([128, 512], F32, tag="tr", name="tq")
                    for i in range(4):
                        c = quad * 4 + i
                        nc.tensor.transpose(
                            tq[0:64, i * 128:(i + 1) * 128],
                            qn[:, c, :], ident)
                    nc.vector.tensor_copy(
                        qT[0:64, quad * 512:(quad + 1) * 512], tq[0:64, :])
                nc.sync.dma_start(out=qT[64:128], in_=qT[0:64])
                return qT, kT, vt

            def body():
                ctx = {0: emit_preamble(0)}

                for h in range(heads):
                    qT, kT, vt = ctx.pop(h)
                    ps_t, et_t, pv_t = {}, {}, {}
                    if SKIP_EXP:
                        et_fix = exps.tile([128, IG], F32R, tag="et",
                                           name="et_fix")
                        nc.vector.tensor_copy(
                            et_fix,
                            vt.rearrange("p a b -> p (a b)")[:, 0:IG])

                    def emit_qk(m):
                        # half-step m: g = m//(2*NG... flat: step n = m//2
                        n, half = divmod(m, 2)
                        g, cc = divmod(n, NJ // 2)
                        ps = qkps.tile([128, IG], F32, tag="ps", name="ps")
                        ps_t[m] = ps
                        nc.tensor.matmul(
                            ps,
                            kT[half * 64:half * 64 + 64,
                               cc * 128:(cc + 1) * 128],
                            qT[half * 64:half * 64 + 64,
                               g * IG:(g + 1) * IG],
                            start=True, stop=True,
                            tile_position=(half * 64, 0))

                    def emit_exp(m):
                        if SKIP_EXP:
                            ps_t.pop(m)
                            et_t[m] = et_fix
                            return
                        ps = ps_t.pop(m)
                        et = exps.tile([128, IG], F32R, tag="et", name="et")
                        et_t[m] = et
                        if acts[m % N_HALVES]:
                            nc.scalar.activation(et, ps, EXP, scale=0.125)
                        else:
                            u = uwp.tile([128, IG], F32, tag="u", name="u")
                            w = uwp.tile([128, IG], F32, tag="w", name="w")
                            nc.vector.tensor_scalar(
                                u.bitcast(I32), ps, A_SCALE, B1_CONST,
                                ALU.mult, ALU.add)
                            eng = nc.gpsimd if USE_POOL_EXP else nc.vector
                            eng.tensor_scalar_sub(
                                w.bitcast(I32), u.bitcast(I32), D_SUB)
                            eng.tensor_tensor(et, u, w, ALU.add)

                    def emit_pv(m):
                        g = m // (2 * (NJ // 2))
                        c = m % NJ          # j-chunk index
                        if c == 0:
                            pv_t[g] = pvps.tile(
                                [80, IG], F32, tag="pv", name="pv")
                        et = et_t.pop(m)
                        nc.tensor.matmul(
                            pv_t[g],
                            vt[:, c, :],
                            et,
                            start=(c == 0), stop=(c == NJ - 1))

                    def emit_epilogue(g):
                        pv = pv_t.pop(g)
                        if USE_DMA_TR:
                            og = osb.tile([80, IG], BF16, tag="og")
                            nc.vector.tensor_copy(og, pv)
                            ot = osb.tile([128, NT, 80], BF16, tag="ot")
                            for t in range(NT):
                                nc.sync.dma_start_transpose(
                                    ot[:, t, :], og[:, t * 128:(t + 1) * 128])
                            rcin = ot[:, :, D:D + 1]
                            ooin = ot[:, :, 0:D]
                        else:
                            og = osb.tile([80, IG], F32R, tag="og")
                            nc.vector.tensor_copy(og, pv)
                            ot = trin.tile([128, NT, 80], F32R, tag="tr",
                                           name="ot")
                            for t in range(NT):
                                nc.tensor.transpose(
                                    ot[:, t, :],
                                    og[:, t * 128:(t + 1) * 128],
                                    identr[0:80, 0:80])
                            rcin = ot[:, :, D:D + 1].bitcast(F32)
                            ooin = ot[:, :, 0:D].bitcast(F32)
                        rc = osb.tile([128, NT, 1], F32, tag="rc")
                        nc.vector.reciprocal(rc, rcin)
                        oo = osb.tile([128, NT, D], F32, tag="oo")
                        nc.vector.tensor_tensor(
                            oo, ooin,
                            rc.broadcast_to([128, NT, D]), ALU.mult)
                        nc.sync.dma_start(
                            out=o_d[h, g * IG:(g + 1) * IG, :].rearrange(
                                "(t p) d -> p t d", p=128),
                            in_=oo)

                    # software pipeline over cc-PAIRS: the two row-tiled QK
                    # halves are emitted adjacently so they co-execute in the
                    # PE array (row-groups 0-1 vs 2-3; measured ~2x).
                    NP = NH // 2
                    for p in range(min(LOOKP, NP)):
                        emit_qk(2 * p)
                        emit_qk(2 * p + 1)
                        if p < LOOKP - 1:
                            emit_exp(2 * p)
                            emit_exp(2 * p + 1)
                    for p in range(NP):
                        emit_pv(2 * p)
                        emit_pv(2 * p + 1)
                        if p + LOOKP < NP:
                            emit_qk(2 * (p + LOOKP))
                            emit_qk(2 * (p + LOOKP) + 1)
                        if p + LOOKP - 1 < NP:
                            emit_exp(2 * (p + LOOKP - 1))
                            emit_exp(2 * (p + LOOKP - 1) + 1)
                        if p == NP // 2 and h + 1 < heads:
                            ctx[h + 1] = emit_preamble(h + 1)
                        if p % (NJ // 2) == NJ // 2 - 1:
                            emit_epilogue(p // (NJ // 2))

            if reps == 1:
                body()
            else:
                with tc.For_i(0, reps, 1):
                    body()

    if walrus_compat:
        _split_sync_waits(nc)
    return nc


_cached_nc = None


def _get_nc():
    global _cached_nc
    if _cached_nc is None:
        _patch_tile_framework()
        _cached_nc = build_nc()
    return _cached_nc


def kernel(q, k, v):
    """Full-shape attention: q/k/v [4, 16, 2048, 64] fp32 -> same shape."""
    from concourse.bass_utils import run_bass_kernel_spmd

    nc = _get_nc()
    q = np.ascontiguousarray(np.asarray(q, dtype=np.float32)).reshape(B * H, S, D)
    k = np.ascontiguousarray(np.asarray(k, dtype=np.float32)).reshape(B * H, S, D)
    v = np.ascontiguousarray(np.asarray(v, dtype=np.float32)).reshape(B * H, S, D)
    hpc = HEADS_PER_CORE
    in_maps = [
        {"q": q[i * hpc:(i + 1) * hpc],
         "k": k[i * hpc:(i + 1) * hpc],
         "v": v[i * hpc:(i + 1) * hpc]}
        for i in range(N_CORES)
    ]
    res = run_bass_kernel_spmd(nc, in_maps, core_ids=list(range(N_CORES)))
    out = np.concatenate([res.results[i]["o"] for i in range(N_CORES)], axis=0)
    return out.reshape(B, H, S, D)


# revision 28
# speedup vs baseline: 1.2408x; 1.0647x over previous
"""TRN2 Bass/Tile kernel for nn_Attention (B=4, H=16, S=2048, D=64, fp32).

Entry point: kernel(q, k, v) -> out, all full-shape [4, 16, 2048, 64] fp32.

Sharding: batch*heads = 64 head-slices, 8 per NeuronCore (data/head
parallel, no cross-core communication). Each core runs the same NEFF on
its own 8 slices via run_bass_kernel_spmd.

v2 design. The v1 baseline was softmax-exp bound: exp on the scalar
(ACT) engine is 1 elem/cycle/partition @1.2GHz = ~218us/core of ACT time
vs ~180us of PE time, measured 379-439us with imperfect overlap. v2:

  - S^T formulation: QK^T row-packed matmuls (two 64-contraction halves
    at tile_position (0,0)/(64,0)) produce S^T[j,i] half-tiles [128,512]
    in PSUM; PV accumulates O_aug^T[65, i] += Vtilde_c^T @ expS^T_c over
    j-chunks (Vtilde = [V | ones]; row 64 = softmax denominator).
  - qT/kT are bf16 (scores' error ~0.3% of sigma -- negligible), built
    with zero PE/DVE work: Pool (gpsimd) converts f32->bf16, then
    XBAR DMA-transposes write each chunk's [64,128] transpose directly
    into the qT/kT layouts. PE runs *only* QK and PV matmuls.
  - exp is split across engines per [128,512] half-step:
      * ACT halves: true Exp activation (scale=1/8 folded in).
      * DVE halves: Schraudolph-style bitcast exponential, sum of two
        affine-int32 terms (2-segment piecewise-linear, rel err ~±1%):
          u  = int32(A*qk + B1)      (DVE tensor_scalar, PSUM->SBUF)
          w  = u - D                 (Pool int sub, SBUF->SBUF)
          et = f32(u) + f32(w)       (Pool tensor_tensor add)
        Per-head numpy sim of the mixed softmax: rel err ~6e-3.
  - software-pipelined emission: QK runs LOOKAHEAD half-steps ahead of
    PV so the in-order PE queue never waits on exp; next head's
    loads/converts/transposes are emitted mid-head.
  - epilogue per i-group: O_aug^T -> bf16 -> XBAR DMA-transpose ->
    [128, 4, 80]; batched DVE reciprocal + broadcast multiply.
  - PSUM: 6 banks of QK half-tiles + 2 banks of PV accumulators.

This container's walrus build rejects sync waits on Drain instructions
and allows at most one sync wait on any other instruction, while Tile
freely attaches several; _patch_tile_framework() + _split_sync_waits()
below rework the exit barrier and hoist excess waits onto injected NOPs.
"""
import sys

if '/opt/trn_rl_repo' not in sys.path:
    sys.path.insert(0, '/opt/trn_rl_repo')

import numpy as np

import concourse.bass as bass
import concourse.tile as tile
from concourse import mybir
from concourse.masks import make_identity
from concourse.vector_clock import ScopedClock

F32 = mybir.dt.float32
F32R = mybir.dt.float32r
BF16 = mybir.dt.bfloat16
I32 = mybir.dt.int32
EXP = mybir.ActivationFunctionType.Exp
ALU = mybir.AluOpType

B, H, S, D = 4, 16, 2048, 64
N_CORES = 8
HEADS_PER_CORE = B * H // N_CORES

# --- exp approximation constants (see module docstring) -------------------
LOG2E = 1.4426950408889634
LA1 = -0.8263                  # tuned: minimax of S(x,la1)+S(x,la2) vs exp
LA2 = -1.3163
A_SCALE = LOG2E * (1 << 23) * 0.125       # folds the 1/sqrt(D) score scale
B1_CONST = (127.0 + LA1) * (1 << 23)
D_SUB = round((LA1 - LA2) * (1 << 23))

# --- engine assignment knobs ---------------------------------------------
N_HALVES = 64           # half-steps per head: (i-groups=4) x (j-pairs=8) x 2
ACT_HALVES = 41         # halves whose exp runs on ACT; rest on DVE+Pool
USE_POOL_EXP = False    # TS2+TT on gpsimd (False: on DVE)
USE_POOL_CVT = True     # preamble vt convert on gpsimd (False: on DVE)
USE_DMA_TR = False      # XBAR DMA transposes (False: PE transposes)
SKIP_EXP = False        # timing-only: PV reads one static et; exp lanes idle


def _assignment():
    """Bresenham-spread booleans: True = ACT half-step."""
    return [
        (t + 1) * ACT_HALVES // N_HALVES - t * ACT_HALVES // N_HALVES == 1
        for t in range(N_HALVES)
    ]


# ---------------------------------------------------------------------------
# Walrus compatibility patches
# ---------------------------------------------------------------------------
_patched = False
_split_counter = [0]


def _patched_multi_engine_barrier(self, engines):
    for e in engines:
        self.engines[e].drain(fusable=False)
    for inst in self._sem_only_all_engine_barrier_insts(f"aeb{self.next_id()}"):
        self.engines[inst.engine].add_instruction(inst)


def _patched_drain_and_barrier(self, tick_clock, wait_clock):
    nop_inst = self.nc.sync.nop(nofuse=True, hint="tile_exit_wait")
    wait_clock.add_sem_waits(
        nop_inst.ins, ScopedClock({None: tick_clock.global_clock})
    )
    self.nc.sync.drain()
    self.nc.all_engine_barrier()
    assert self.sems is not None
    popped = self.nc._tile_sem_poison_stack.pop()
    assert popped is self._sem_poison
    self.nc.clear_and_free_semaphores(list(self.sems.allocated().values()))
    self.nc.all_engine_barrier()


def _patch_tile_framework():
    global _patched
    if _patched:
        return
    bass.Bass.multi_engine_barrier = _patched_multi_engine_barrier
    tile.TileContext._drain_and_barrier = _patched_drain_and_barrier
    _patched = True


def _split_sync_waits(nc):
    """No instruction may carry more than the walrus-supported number of
    sync waits (0 for Drain, 1 otherwise); hoist the rest onto NOPs."""
    for f in nc.m.functions:
        for bb in f.blocks:
            insts = bb.instructions
            if not any(
                i.sync_info is not None
                and len(i.sync_info.on_wait) > (0 if i.opcode == "Drain" else 1)
                for i in insts
            ):
                continue
            out = []
            for inst in insts:
                si = inst.sync_info
                limit = 0 if inst.opcode == "Drain" else 1
                if si is not None and len(si.on_wait) > limit:
                    waits = list(si.on_wait)
                    keep, extra = waits[:limit], waits[limit:]
                    for w in extra:
                        _split_counter[0] += 1
                        nop = mybir.InstNoOp(
                            name=f"waitsplit-{_split_counter[0]}", ins=[], outs=[]
                        )
                        nop.engine = inst.engine
                        nop.sync_info = mybir.SyncInfo(on_wait=[w], on_update=[])
                        out.append(nop)
                    inst.sync_info = mybir.SyncInfo(
                        on_wait=keep, on_update=list(si.on_update)
                    )
                out.append(inst)
            bb.instructions = out


# ---------------------------------------------------------------------------
# Kernel builder
# ---------------------------------------------------------------------------
def build_nc(heads=HEADS_PER_CORE, s=S, reps=1, walrus_compat=True):
    NJ = s // 128           # j (k-row) chunks of 128
    IG = 512                # i (q-row) group width
    NG = s // IG
    NT = IG // 128
    NH = NG * NJ            # half-steps per head (== N_HALVES for s=2048)
    LOOKP = 2               # QK lookahead in cc-pairs

    acts = _assignment()

    nc = bass.Bass(target_bir_lowering=False)
    q_d = nc.dram_tensor("q", [heads, s, D], F32, kind="ExternalInput")
    k_d = nc.dram_tensor("k", [heads, s, D], F32, kind="ExternalInput")
    v_d = nc.dram_tensor("v", [heads, s, D], F32, kind="ExternalInput")
    o_d = nc.dram_tensor("o", [heads, s, D], F32, kind="ExternalOutput")

    with tile.TileContext(nc) as tc:
        with (
            tc.tile_pool(name="qkin", bufs=2) as qkin,
            tc.tile_pool(name="qkT", bufs=2) as qkT,
            tc.tile_pool(name="exps", bufs=8) as exps,
            tc.tile_pool(name="uw", bufs=4) as uwp,
            tc.tile_pool(name="osb", bufs=2) as osb,
            tc.tile_pool(name="singles", bufs=1) as singles,
            tc.tile_pool(name="qkps", bufs=5, space="PSUM") as qkps,
            tc.tile_pool(name="pvps", bufs=2, space="PSUM") as pvps,
            tc.tile_pool(name="trin", bufs=1, space="PSUM") as trin,
        ):
            if not USE_DMA_TR:
                ident = singles.tile([128, 128], F32)
                make_identity(nc, ident)
                identr = singles.tile([128, 128], F32R)
                nc.vector.tensor_copy(identr, ident)
            def emit_preamble(h):
                """Loads + bf16 transposed layouts for head h via Pool
                converts and XBAR DMA-transposes. Returns (qT, kT, vl)."""
                ceng = nc.gpsimd if USE_POOL_CVT else nc.vector
                qn = qkin.tile([128, NJ, D], F32, tag="qn")
                kn = qkin.tile([128, NJ, D], F32, tag="kn")
                nc.sync.dma_start(
                    out=qn, in_=q_d[h].rearrange("(c p) d -> p c d", p=128))
                nc.sync.dma_start(
                    out=kn, in_=k_d[h].rearrange("(c p) d -> p c d", p=128))
                vl = qkin.tile([128, NJ, 80], F32, tag="vl")
                nc.sync.dma_start(
                    out=vl[:, :, 0:D],
                    in_=v_d[h].rearrange("(c p) d -> p c d", p=128))
                nc.vector.memset(vl[:, :, D:D + 1], 1.0)
                nc.vector.memset(vl[:, :, D + 1:80], 0.0)
                vt = qkin.tile([128, NJ, 80], F32R, tag="vt")
                ceng.tensor_copy(vt, vl)

                kT = qkT.tile([128, s // 2], BF16, tag="kT")
                if USE_DMA_TR:
                    qTp = qkT.tile([128, s // 2], BF16, tag="qTp")
                    # XBAR-transpose chunk PAIRS ([128,128] slabs; free dim
                    # must be a multiple of 128). Output rows 0:64 = even
                    # chunk, 64:128 = odd -- exactly the kT pair layout.
                    qb = qkin.tile([128, NJ, D], BF16, tag="qb")
                    kb = qkin.tile([128, NJ, D], BF16, tag="kb")
                    ceng.tensor_copy(qb, qn)
                    ceng.tensor_copy(kb, kn)
                    for pp in range(NJ // 2):
                        nc.sync.dma_start_transpose(
                            kT[:, pp * 128:(pp + 1) * 128],
                            kb[:, 2 * pp:2 * pp + 2, :].rearrange(
                                "p a b -> p (a b)"))
                    for pp in range(NJ // 2):
                        nc.sync.dma_start_transpose(
                            qTp[:, pp * 128:(pp + 1) * 128],
                            qb[:, 2 * pp:2 * pp + 2, :].rearrange(
                                "p a b -> p (a b)"))
                    # qTp[a*64+d, pp*128+p] = q[(2pp+a)*128+p, d]; reshuffle
                    # into qT[d, c*128+p] = q[c*128+p, d], both halves.
                    qT = qkT.tile([128, s], BF16, tag="qT")
                    nc.sync.dma_start(
                        out=qT[0:64].rearrange(
                            "p (a c) -> p a c", a=NJ // 2)[:, :, 0:128],
                        in_=qTp[0:64].rearrange("p (a c) -> p a c", a=NJ // 2))
                    nc.sync.dma_start(
                        out=qT[0:64].rearrange(
                            "p (a c) -> p a c", a=NJ // 2)[:, :, 128:256],
                        in_=qTp[64:128].rearrange(
                            "p (a c) -> p a c", a=NJ // 2))
                    nc.sync.dma_start(out=qT[64:128], in_=qT[0:64])
                    return qT, kT, vt
                # PE path: pair-transposes ([128,2,64] -> [128,128] slabs,
                # fp32 at 2 c/row), DVE copies convert PSUM f32 -> bf16.
                qTp = qkT.tile([128, s // 2], BF16, tag="qTp")
                for src_, dst in ((kn, kT), (qn, qTp)):
                    for hf in range(2):
                        tk = trin.tile([128, 512], F32, tag="tr", name="tk")
                        for i in range(4):
                            pp = hf * 4 + i
                            nc.tensor.transpose(
                                tk[:, i * 128:(i + 1) * 128],
                                src_[:, 2 * pp:2 * pp + 2, :].rearrange(
                                    "p a b -> p (a b)"),
                                ident)
                        nc.vector.tensor_copy(
                            dst[:, hf * 512:(hf + 1) * 512], tk)
                # qTp[a*64+d, pp*128+p] = q[(2pp+a)*128+p, d]; reshuffle into
                # qT[d, c*128+p] = q[c*128+p, d], both partition halves.
                qT = qkT.tile([128, s], BF16, tag="qT")
                nc.sync.dma_start(
                    out=qT[0:64].rearrange(
                        "p (a c) -> p a c", a=NJ // 2)[:, :, 0:128],
                    in_=qTp[0:64].rearrange("p (a c) -> p a c", a=NJ // 2))
                nc.sync.dma_start(
                    out=qT[0:64].rearrange(
                        "p (a c) -> p a c", a=NJ // 2)[:, :, 128:256],
                    in_=qTp[64:128].rearrange("p (a c) -> p a c", a=NJ // 2))
                nc.sync.dma_start(out=qT[64:128], in_=qT[0:64])
                return qT, kT, vt

            def body():
                ctx = {0: emit_preamble(0)}

                for h in range(heads):
                    qT, kT, vt = ctx.pop(h)
                    ps_t, et_t, pv_t = {}, {}, {}
                    if SKIP_EXP:
                        et_fix = exps.tile([128, IG], F32R, tag="et",
                                           name="et_fix")
                        nc.vector.tensor_copy(
                            et_fix,
                            vt.rearrange("p a b -> p (a b)")[:, 0:IG])

                    def emit_qk(m):
                        # half-step m: g = m//(2*NG... flat: step n = m//2
                        n, half = divmod(m, 2)
                        g, cc = divmod(n, NJ // 2)
                        ps = qkps.tile([128, IG], F32, tag="ps", name="ps")
                        ps_t[m] = ps
                        nc.tensor.matmul(
                            ps,
                            kT[half * 64:half * 64 + 64,
                               cc * 128:(cc + 1) * 128],
                            qT[half * 64:half * 64 + 64,
                               g * IG:(g + 1) * IG],
                            start=True, stop=True,
                            tile_position=(half * 64, 0))

                    def emit_exp(m):
                        if SKIP_EXP:
                            ps_t.pop(m)
                            et_t[m] = et_fix
                            return
                        ps = ps_t.pop(m)
                        et = exps.tile([128, IG], F32R, tag="et", name="et")
                        et_t[m] = et
                        if acts[m % N_HALVES]:
                            nc.scalar.activation(et, ps, EXP, scale=0.125)
                        else:
                            u = uwp.tile([128, IG], F32, tag="u", name="u")
                            w = uwp.tile([128, IG], F32, tag="w", name="w")
                            nc.vector.tensor_scalar(
                                u.bitcast(I32), ps, A_SCALE, B1_CONST,
                                ALU.mult, ALU.add)
                            eng = nc.gpsimd if USE_POOL_EXP else nc.vector
                            eng.tensor_scalar_sub(
                                w.bitcast(I32), u.bitcast(I32), D_SUB)
                            eng.tensor_tensor(et, u, w, ALU.add)

                    def emit_pv(m):
                        g = m // (2 * (NJ // 2))
                        c = m % NJ          # j-chunk index
                        if c == 0:
                            pv_t[g] = pvps.tile(
                                [80, IG], F32, tag="pv", name="pv")
                        et = et_t.pop(m)
                        nc.tensor.matmul(
                            pv_t[g],
                            vt[:, c, :],
                            et,
                            start=(c == 0), stop=(c == NJ - 1))

                    def emit_epilogue(g):
                        pv = pv_t.pop(g)
                        if USE_DMA_TR:
                            og = osb.tile([80, IG], BF16, tag="og")
                            nc.vector.tensor_copy(og, pv)
                            ot = osb.tile([128, NT, 80], BF16, tag="ot")
                            for t in range(NT):
                                nc.sync.dma_start_transpose(
                                    ot[:, t, :], og[:, t * 128:(t + 1) * 128])
                            rcin = ot[:, :, D:D + 1]
                            ooin = ot[:, :, 0:D]
                        else:
                            og = osb.tile([80, IG], F32R, tag="og")
                            nc.vector.tensor_copy(og, pv)
                            ot = trin.tile([128, NT, 80], F32R, tag="tr",
                                           name="ot")
                            for t in range(NT):
                                nc.tensor.transpose(
                                    ot[:, t, :],
                                    og[:, t * 128:(t + 1) * 128],
                                    identr[0:80, 0:80])
                            rcin = ot[:, :, D:D + 1].bitcast(F32)
                            ooin = ot[:, :, 0:D].bitcast(F32)
                        rc = osb.tile([128, NT, 1], F32, tag="rc")
                        nc.vector.reciprocal(rc, rcin)
                        oo = osb.tile([128, NT, D], F32, tag="oo")
                        nc.vector.tensor_tensor(
                            oo, ooin,
                            rc.broadcast_to([128, NT, D]), ALU.mult)
                        nc.sync.dma_start(
                            out=o_d[h, g * IG:(g + 1) * IG, :].rearrange(
                                "(t p) d -> p t d", p=128),
                            in_=oo)

                    # software pipeline over cc-PAIRS: the two row-tiled QK
                    # halves are emitted adjacently so they co-execute in the
                    # PE array (row-groups 0-1 vs 2-3; measured ~2x).
                    NP = NH // 2
                    for p in range(min(LOOKP, NP)):
                        emit_qk(2 * p)
                        emit_qk(2 * p + 1)
                        if p < LOOKP - 1:
                            emit_exp(2 * p)
                            emit_exp(2 * p + 1)
                    for p in range(NP):
                        emit_pv(2 * p)
                        emit_pv(2 * p + 1)
                        if p + LOOKP < NP:
                            emit_qk(2 * (p + LOOKP))
                            emit_qk(2 * (p + LOOKP) + 1)
                        if p + LOOKP - 1 < NP:
                            emit_exp(2 * (p + LOOKP - 1))
                            emit_exp(2 * (p + LOOKP - 1) + 1)
                        if p == NP // 2 and h + 1 < heads:
                            ctx[h + 1] = emit_preamble(h + 1)
                        if p % (NJ // 2) == NJ // 2 - 1:
                            emit_epilogue(p // (NJ // 2))

            if reps == 1:
                body()
            else:
                with tc.For_i(0, reps, 1):
                    body()

    if walrus_compat:
        _split_sync_waits(nc)
    return nc


_cached_nc = None


def _get_nc():
    global _cached_nc
    if _cached_nc is None:
        _patch_tile_framework()
        _cached_nc = build_nc()
    return _cached_nc


def kernel(q, k, v):
    """Full-shape attention: q/k/v [4, 16, 2048, 64] fp32 -> same shape."""
    from concourse.bass_utils import run_bass_kernel_spmd

    nc = _get_nc()
    q = np.ascontiguousarray(np.asarray(q, dtype=np.float32)).reshape(B * H, S, D)
    k = np.ascontiguousarray(np.asarray(k, dtype=np.float32)).reshape(B * H, S, D)
    v = np.ascontiguousarray(np.asarray(v, dtype=np.float32)).reshape(B * H, S, D)
    hpc = HEADS_PER_CORE
    in_maps = [
        {"q": q[i * hpc:(i + 1) * hpc],
         "k": k[i * hpc:(i + 1) * hpc],
         "v": v[i * hpc:(i + 1) * hpc]}
        for i in range(N_CORES)
    ]
    res = run_bass_kernel_spmd(nc, in_maps, core_ids=list(range(N_CORES)))
    out = np.concatenate([res.results[i]["o"] for i in range(N_CORES)], axis=0)
    return out.reshape(B, H, S, D)


# revision 29
# speedup vs baseline: 1.2494x; 1.0069x over previous
"""TRN2 Bass/Tile kernel for nn_Attention (B=4, H=16, S=2048, D=64, fp32).

Entry point: kernel(q, k, v) -> out, all full-shape [4, 16, 2048, 64] fp32.

Sharding: batch*heads = 64 head-slices, 8 per NeuronCore (data/head
parallel, no cross-core communication). Each core runs the same NEFF on
its own 8 slices via run_bass_kernel_spmd.

v2 design. The v1 baseline was softmax-exp bound: exp on the scalar
(ACT) engine is 1 elem/cycle/partition @1.2GHz = ~218us/core of ACT time
vs ~180us of PE time, measured 379-439us with imperfect overlap. v2:

  - S^T formulation: QK^T row-packed matmuls (two 64-contraction halves
    at tile_position (0,0)/(64,0)) produce S^T[j,i] half-tiles [128,512]
    in PSUM; PV accumulates O_aug^T[65, i] += Vtilde_c^T @ expS^T_c over
    j-chunks (Vtilde = [V | ones]; row 64 = softmax denominator).
  - qT/kT are bf16 (scores' error ~0.3% of sigma -- negligible), built
    with zero PE/DVE work: Pool (gpsimd) converts f32->bf16, then
    XBAR DMA-transposes write each chunk's [64,128] transpose directly
    into the qT/kT layouts. PE runs *only* QK and PV matmuls.
  - exp is split across engines per [128,512] half-step:
      * ACT halves: true Exp activation (scale=1/8 folded in).
      * DVE halves: Schraudolph-style bitcast exponential, sum of two
        affine-int32 terms (2-segment piecewise-linear, rel err ~±1%):
          u  = int32(A*qk + B1)      (DVE tensor_scalar, PSUM->SBUF)
          w  = u - D                 (Pool int sub, SBUF->SBUF)
          et = f32(u) + f32(w)       (Pool tensor_tensor add)
        Per-head numpy sim of the mixed softmax: rel err ~6e-3.
  - software-pipelined emission: QK runs LOOKAHEAD half-steps ahead of
    PV so the in-order PE queue never waits on exp; next head's
    loads/converts/transposes are emitted mid-head.
  - epilogue per i-group: O_aug^T -> bf16 -> XBAR DMA-transpose ->
    [128, 4, 80]; batched DVE reciprocal + broadcast multiply.
  - PSUM: 6 banks of QK half-tiles + 2 banks of PV accumulators.

This container's walrus build rejects sync waits on Drain instructions
and allows at most one sync wait on any other instruction, while Tile
freely attaches several; _patch_tile_framework() + _split_sync_waits()
below rework the exit barrier and hoist excess waits onto injected NOPs.
"""
import sys

if '/opt/trn_rl_repo' not in sys.path:
    sys.path.insert(0, '/opt/trn_rl_repo')

import numpy as np

import concourse.bass as bass
import concourse.tile as tile
from concourse import mybir
from concourse.masks import make_identity
from concourse.vector_clock import ScopedClock

F32 = mybir.dt.float32
F32R = mybir.dt.float32r
BF16 = mybir.dt.bfloat16
I32 = mybir.dt.int32
EXP = mybir.ActivationFunctionType.Exp
ALU = mybir.AluOpType

B, H, S, D = 4, 16, 2048, 64
N_CORES = 8
HEADS_PER_CORE = B * H // N_CORES

# --- exp approximation constants (see module docstring) -------------------
LOG2E = 1.4426950408889634
LA1 = -0.8263                  # tuned: minimax of S(x,la1)+S(x,la2) vs exp
LA2 = -1.3163
A_SCALE = LOG2E * (1 << 23) * 0.125       # folds the 1/sqrt(D) score scale
B1_CONST = (127.0 + LA1) * (1 << 23)
D_SUB = round((LA1 - LA2) * (1 << 23))

# --- engine assignment knobs ---------------------------------------------
N_HALVES = 64           # half-steps per head: (i-groups=4) x (j-pairs=8) x 2
ACT_HALVES = 41         # halves whose exp runs on ACT; rest on DVE+Pool
USE_POOL_EXP = False    # TS2+TT on gpsimd (False: on DVE)
USE_POOL_CVT = True     # preamble vt convert on gpsimd (False: on DVE)
USE_DMA_TR = False      # XBAR DMA transposes (False: PE transposes)
SKIP_EXP = False        # timing-only: PV reads one static et; exp lanes idle


def _assignment():
    """Bresenham-spread booleans: True = ACT half-step."""
    return [
        (t + 1) * ACT_HALVES // N_HALVES - t * ACT_HALVES // N_HALVES == 1
        for t in range(N_HALVES)
    ]


# ---------------------------------------------------------------------------
# Walrus compatibility patches
# ---------------------------------------------------------------------------
_patched = False
_split_counter = [0]


def _patched_multi_engine_barrier(self, engines):
    for e in engines:
        self.engines[e].drain(fusable=False)
    for inst in self._sem_only_all_engine_barrier_insts(f"aeb{self.next_id()}"):
        self.engines[inst.engine].add_instruction(inst)


def _patched_drain_and_barrier(self, tick_clock, wait_clock):
    nop_inst = self.nc.sync.nop(nofuse=True, hint="tile_exit_wait")
    wait_clock.add_sem_waits(
        nop_inst.ins, ScopedClock({None: tick_clock.global_clock})
    )
    self.nc.sync.drain()
    self.nc.all_engine_barrier()
    assert self.sems is not None
    popped = self.nc._tile_sem_poison_stack.pop()
    assert popped is self._sem_poison
    self.nc.clear_and_free_semaphores(list(self.sems.allocated().values()))
    self.nc.all_engine_barrier()


def _patch_tile_framework():
    global _patched
    if _patched:
        return
    bass.Bass.multi_engine_barrier = _patched_multi_engine_barrier
    tile.TileContext._drain_and_barrier = _patched_drain_and_barrier
    _patched = True


def _split_sync_waits(nc):
    """No instruction may carry more than the walrus-supported number of
    sync waits (0 for Drain, 1 otherwise); hoist the rest onto NOPs."""
    for f in nc.m.functions:
        for bb in f.blocks:
            insts = bb.instructions
            if not any(
                i.sync_info is not None
                and len(i.sync_info.on_wait) > (0 if i.opcode == "Drain" else 1)
                for i in insts
            ):
                continue
            out = []
            for inst in insts:
                si = inst.sync_info
                limit = 0 if inst.opcode == "Drain" else 1
                if si is not None and len(si.on_wait) > limit:
                    waits = list(si.on_wait)
                    keep, extra = waits[:limit], waits[limit:]
                    for w in extra:
                        _split_counter[0] += 1
                        nop = mybir.InstNoOp(
                            name=f"waitsplit-{_split_counter[0]}", ins=[], outs=[]
                        )
                        nop.engine = inst.engine
                        nop.sync_info = mybir.SyncInfo(on_wait=[w], on_update=[])
                        out.append(nop)
                    inst.sync_info = mybir.SyncInfo(
                        on_wait=keep, on_update=list(si.on_update)
                    )
                out.append(inst)
            bb.instructions = out


# ---------------------------------------------------------------------------
# Kernel builder
# ---------------------------------------------------------------------------
def build_nc(heads=HEADS_PER_CORE, s=S, reps=1, walrus_compat=True):
    NJ = s // 128           # j (k-row) chunks of 128
    IG = 512                # i (q-row) group width
    NG = s // IG
    NT = IG // 128
    NH = NG * NJ            # half-steps per head (== N_HALVES for s=2048)
    LOOKP = 2               # QK lookahead in cc-pairs

    acts = _assignment()

    nc = bass.Bass(target_bir_lowering=False)
    q_d = nc.dram_tensor("q", [heads, s, D], F32, kind="ExternalInput")
    k_d = nc.dram_tensor("k", [heads, s, D], F32, kind="ExternalInput")
    v_d = nc.dram_tensor("v", [heads, s, D], F32, kind="ExternalInput")
    o_d = nc.dram_tensor("o", [heads, s, D], F32, kind="ExternalOutput")

    with tile.TileContext(nc) as tc:
        with (
            tc.tile_pool(name="qkin", bufs=2) as qkin,
            tc.tile_pool(name="qkT", bufs=2) as qkT,
            tc.tile_pool(name="exps", bufs=10) as exps,
            tc.tile_pool(name="uw", bufs=6) as uwp,
            tc.tile_pool(name="osb", bufs=2) as osb,
            tc.tile_pool(name="singles", bufs=1) as singles,
            tc.tile_pool(name="qkps", bufs=5, space="PSUM") as qkps,
            tc.tile_pool(name="pvps", bufs=2, space="PSUM") as pvps,
            tc.tile_pool(name="trin", bufs=1, space="PSUM") as trin,
        ):
            if not USE_DMA_TR:
                ident = singles.tile([128, 128], F32)
                make_identity(nc, ident)
                identr = singles.tile([128, 128], F32R)
                nc.vector.tensor_copy(identr, ident)
            def emit_preamble(h):
                """Loads + bf16 transposed layouts for head h via Pool
                converts and XBAR DMA-transposes. Returns (qT, kT, vl)."""
                ceng = nc.gpsimd if USE_POOL_CVT else nc.vector
                qn = qkin.tile([128, NJ, D], F32, tag="qn")
                kn = qkin.tile([128, NJ, D], F32, tag="kn")
                nc.sync.dma_start(
                    out=qn, in_=q_d[h].rearrange("(c p) d -> p c d", p=128))
                nc.sync.dma_start(
                    out=kn, in_=k_d[h].rearrange("(c p) d -> p c d", p=128))
                vl = qkin.tile([128, NJ, 80], F32, tag="vl")
                nc.sync.dma_start(
                    out=vl[:, :, 0:D],
                    in_=v_d[h].rearrange("(c p) d -> p c d", p=128))
                nc.vector.memset(vl[:, :, D:D + 1], 1.0)
                nc.vector.memset(vl[:, :, D + 1:80], 0.0)
                vt = qkin.tile([128, NJ, 80], F32R, tag="vt")
                ceng.tensor_copy(vt, vl)

                kT = qkT.tile([128, s // 2], BF16, tag="kT")
                if USE_DMA_TR:
                    qTp = qkT.tile([128, s // 2], BF16, tag="qTp")
                    # XBAR-transpose chunk PAIRS ([128,128] slabs; free dim
                    # must be a multiple of 128). Output rows 0:64 = even
                    # chunk, 64:128 = odd -- exactly the kT pair layout.
                    qb = qkin.tile([128, NJ, D], BF16, tag="qb")
                    kb = qkin.tile([128, NJ, D], BF16, tag="kb")
                    ceng.tensor_copy(qb, qn)
                    ceng.tensor_copy(kb, kn)
                    for pp in range(NJ // 2):
                        nc.sync.dma_start_transpose(
                            kT[:, pp * 128:(pp + 1) * 128],
                            kb[:, 2 * pp:2 * pp + 2, :].rearrange(
                                "p a b -> p (a b)"))
                    for pp in range(NJ // 2):
                        nc.sync.dma_start_transpose(
                            qTp[:, pp * 128:(pp + 1) * 128],
                            qb[:, 2 * pp:2 * pp + 2, :].rearrange(
                                "p a b -> p (a b)"))
                    # qTp[a*64+d, pp*128+p] = q[(2pp+a)*128+p, d]; reshuffle
                    # into qT[d, c*128+p] = q[c*128+p, d], both halves.
                    qT = qkT.tile([128, s], BF16, tag="qT")
                    nc.sync.dma_start(
                        out=qT[0:64].rearrange(
                            "p (a c) -> p a c", a=NJ // 2)[:, :, 0:128],
                        in_=qTp[0:64].rearrange("p (a c) -> p a c", a=NJ // 2))
                    nc.sync.dma_start(
                        out=qT[0:64].rearrange(
                            "p (a c) -> p a c", a=NJ // 2)[:, :, 128:256],
                        in_=qTp[64:128].rearrange(
                            "p (a c) -> p a c", a=NJ // 2))
                    nc.sync.dma_start(out=qT[64:128], in_=qT[0:64])
                    return qT, kT, vt
                # PE path: pair-transposes ([128,2,64] -> [128,128] slabs,
                # fp32 at 2 c/row), DVE copies convert PSUM f32 -> bf16.
                qTp = qkT.tile([128, s // 2], BF16, tag="qTp")
                for src_, dst in ((kn, kT), (qn, qTp)):
                    for hf in range(2):
                        tk = trin.tile([128, 512], F32, tag="tr", name="tk")
                        for i in range(4):
                            pp = hf * 4 + i
                            nc.tensor.transpose(
                                tk[:, i * 128:(i + 1) * 128],
                                src_[:, 2 * pp:2 * pp + 2, :].rearrange(
                                    "p a b -> p (a b)"),
                                ident)
                        nc.vector.tensor_copy(
                            dst[:, hf * 512:(hf + 1) * 512], tk)
                # qTp[a*64+d, pp*128+p] = q[(2pp+a)*128+p, d]; reshuffle into
                # qT[d, c*128+p] = q[c*128+p, d], both partition halves.
                qT = qkT.tile([128, s], BF16, tag="qT")
                nc.sync.dma_start(
                    out=qT[0:64].rearrange(
                        "p (a c) -> p a c", a=NJ // 2)[:, :, 0:128],
                    in_=qTp[0:64].rearrange("p (a c) -> p a c", a=NJ // 2))
                nc.sync.dma_start(
                    out=qT[0:64].rearrange(
                        "p (a c) -> p a c", a=NJ // 2)[:, :, 128:256],
                    in_=qTp[64:128].rearrange("p (a c) -> p a c", a=NJ // 2))
                nc.sync.dma_start(out=qT[64:128], in_=qT[0:64])
                return qT, kT, vt

            def body():
                ctx = {0: emit_preamble(0)}

                for h in range(heads):
                    qT, kT, vt = ctx.pop(h)
                    ps_t, et_t, pv_t = {}, {}, {}
                    if SKIP_EXP:
                        et_fix = exps.tile([128, IG], F32R, tag="et",
                                           name="et_fix")
                        nc.vector.tensor_copy(
                            et_fix,
                            vt.rearrange("p a b -> p (a b)")[:, 0:IG])

                    def emit_qk(m):
                        # half-step m: g = m//(2*NG... flat: step n = m//2
                        n, half = divmod(m, 2)
                        g, cc = divmod(n, NJ // 2)
                        ps = qkps.tile([128, IG], F32, tag="ps", name="ps")
                        ps_t[m] = ps
                        nc.tensor.matmul(
                            ps,
                            kT[half * 64:half * 64 + 64,
                               cc * 128:(cc + 1) * 128],
                            qT[half * 64:half * 64 + 64,
                               g * IG:(g + 1) * IG],
                            start=True, stop=True,
                            tile_position=(half * 64, 0))

                    def emit_exp(m):
                        if SKIP_EXP:
                            ps_t.pop(m)
                            et_t[m] = et_fix
                            return
                        ps = ps_t.pop(m)
                        et = exps.tile([128, IG], F32R, tag="et", name="et")
                        et_t[m] = et
                        if acts[m % N_HALVES]:
                            nc.scalar.activation(et, ps, EXP, scale=0.125)
                        else:
                            u = uwp.tile([128, IG], F32, tag="u", name="u")
                            w = uwp.tile([128, IG], F32, tag="w", name="w")
                            nc.vector.tensor_scalar(
                                u.bitcast(I32), ps, A_SCALE, B1_CONST,
                                ALU.mult, ALU.add)
                            eng = nc.gpsimd if USE_POOL_EXP else nc.vector
                            eng.tensor_scalar_sub(
                                w.bitcast(I32), u.bitcast(I32), D_SUB)
                            eng.tensor_tensor(et, u, w, ALU.add)

                    def emit_pv(m):
                        g = m // (2 * (NJ // 2))
                        c = m % NJ          # j-chunk index
                        if c == 0:
                            pv_t[g] = pvps.tile(
                                [80, IG], F32, tag="pv", name="pv")
                        et = et_t.pop(m)
                        nc.tensor.matmul(
                            pv_t[g],
                            vt[:, c, :],
                            et,
                            start=(c == 0), stop=(c == NJ - 1))

                    def emit_epilogue(g):
                        pv = pv_t.pop(g)
                        if USE_DMA_TR:
                            og = osb.tile([80, IG], BF16, tag="og")
                            nc.vector.tensor_copy(og, pv)
                            ot = osb.tile([128, NT, 80], BF16, tag="ot")
                            for t in range(NT):
                                nc.sync.dma_start_transpose(
                                    ot[:, t, :], og[:, t * 128:(t + 1) * 128])
                            rcin = ot[:, :, D:D + 1]
                            ooin = ot[:, :, 0:D]
                        else:
                            og = osb.tile([80, IG], F32R, tag="og")
                            nc.vector.tensor_copy(og, pv)
                            ot = trin.tile([128, NT, 80], F32R, tag="tr",
                                           name="ot")
                            for t in range(NT):
                                nc.tensor.transpose(
                                    ot[:, t, :],
                                    og[:, t * 128:(t + 1) * 128],
                                    identr[0:80, 0:80])
                            rcin = ot[:, :, D:D + 1].bitcast(F32)
                            ooin = ot[:, :, 0:D].bitcast(F32)
                        rc = osb.tile([128, NT, 1], F32, tag="rc")
                        nc.vector.reciprocal(rc, rcin)
                        oo = osb.tile([128, NT, D], F32, tag="oo")
                        nc.vector.tensor_tensor(
                            oo, ooin,
                            rc.broadcast_to([128, NT, D]), ALU.mult)
                        nc.sync.dma_start(
                            out=o_d[h, g * IG:(g + 1) * IG, :].rearrange(
                                "(t p) d -> p t d", p=128),
                            in_=oo)

                    # software pipeline over cc-PAIRS: the two row-tiled QK
                    # halves are emitted adjacently so they co-execute in the
                    # PE array (row-groups 0-1 vs 2-3; measured ~2x).
                    NP = NH // 2
                    for p in range(min(LOOKP, NP)):
                        emit_qk(2 * p)
                        emit_qk(2 * p + 1)
                        if p < LOOKP - 1:
                            emit_exp(2 * p)
                            emit_exp(2 * p + 1)
                    for p in range(NP):
                        emit_pv(2 * p)
                        emit_pv(2 * p + 1)
                        if p + LOOKP < NP:
                            emit_qk(2 * (p + LOOKP))
                            emit_qk(2 * (p + LOOKP) + 1)
                        if p + LOOKP - 1 < NP:
                            emit_exp(2 * (p + LOOKP - 1))
                            emit_exp(2 * (p + LOOKP - 1) + 1)
                        if p == 6 and h + 1 < heads:
                            ctx[h + 1] = emit_preamble(h + 1)
                        if p % (NJ // 2) == NJ // 2 - 1:
                            emit_epilogue(p // (NJ // 2))

            if reps == 1:
                body()
            else:
                with tc.For_i(0, reps, 1):
                    body()

    if walrus_compat:
        _split_sync_waits(nc)
    return nc


_cached_nc = None


def _get_nc():
    global _cached_nc
    if _cached_nc is None:
        _patch_tile_framework()
        _cached_nc = build_nc()
    return _cached_nc


def kernel(q, k, v):
    """Full-shape attention: q/k/v [4, 16, 2048, 64] fp32 -> same shape."""
    from concourse.bass_utils import run_bass_kernel_spmd

    nc = _get_nc()
    q = np.ascontiguousarray(np.asarray(q, dtype=np.float32)).reshape(B * H, S, D)
    k = np.ascontiguousarray(np.asarray(k, dtype=np.float32)).reshape(B * H, S, D)
    v = np.ascontiguousarray(np.asarray(v, dtype=np.float32)).reshape(B * H, S, D)
    hpc = HEADS_PER_CORE
    in_maps = [
        {"q": q[i * hpc:(i + 1) * hpc],
         "k": k[i * hpc:(i + 1) * hpc],
         "v": v[i * hpc:(i + 1) * hpc]}
        for i in range(N_CORES)
    ]
    res = run_bass_kernel_spmd(nc, in_maps, core_ids=list(range(N_CORES)))
    out = np.concatenate([res.results[i]["o"] for i in range(N_CORES)], axis=0)
    return out.reshape(B, H, S, D)
